# revision 30
# baseline (speedup 1.0000x reference)
"""DSAttention (de-stationary causal attention) Trainium2 Bass kernel.

Problem: B=4, L=S=2048, H=8, E=D=64, fp32.
  scores = (Q K^T) * tau_b + delta_b[s]      [B,H,L,S]
  A      = softmax(0.125 * scores) causal-masked
  out    = A V                               [B,L,H,D]

Sharding: the 32 (b,h) pairs are split 4-per-core across 8 NeuronCores
(data + head parallel). Each core runs an identical SPMD Bass program on
its own 4 pairs; no collectives are needed.

Per-core algorithm (per pair):
  - Scores are computed TRANSPOSED, S^T[s, l], one 128-row s-chunk at a
    time: matmul(lhsT=k^T[e, s-chunk] stationary, rhs=q^T[e, l] moving).
  - exp is a single fused ACT pass: exp(scale_ap * qk + bias_ap) where
    scale_ap = 0.125*tau (broadcast to 128 partitions once via a tiny
    matmul) and bias_ap = 0.125*delta^T[s] per-partition column.
  - The causal mask only affects the diagonal 128x128 block of each
    chunk; it is applied AFTER exp as a triangle zeroing via
    gpsimd.affine_select (keeps the hot ACT/DVE engines free).
  - AV uses A^T chunks (bf16) as the 128x128 stationary operand and an
    augmented moving operand [v | 1] of 65 columns, so column 64 of the
    PSUM accumulator is the softmax denominator. Output tiles come out
    in natural [l, d] layout; DVE computes reciprocal + scale.
"""

import numpy as np

try:
    import concourse.bass as bass
except ImportError:  # toolchain not on default path
    import sys

    sys.path.insert(0, "/opt/trn_rl_repo")
    import concourse.bass as bass

import concourse.mybir as mybir
import concourse.tile as tile
from concourse import bacc
from concourse.bass_utils import run_bass_kernel_spmd

B, L, H, E, D = 4, 2048, 8, 64, 64
NCORES = 8
PAIRS = B * H            # 32 (b,h) pairs
PPC = PAIRS // NCORES    # 4 pairs per core
NT = L // 128            # 16 s-chunks / l-tiles per pair
MMW = 512                # max moving width per fp32-out matmul (1 PSUM bank)

F32 = mybir.dt.float32
BF16 = mybir.dt.bfloat16

import os as _os

CFG = {
    "PIECE": int(_os.environ.get("DSATT_PIECE", "1024")),
    "PS_BUFS": int(_os.environ.get("DSATT_PS_BUFS", "3")),
    "JBATCH": int(_os.environ.get("DSATT_JBATCH", "7")),
    "QSPLIT": int(_os.environ.get("DSATT_QSPLIT", "8")),  # first pair-pair
    "QSPLIT2": int(_os.environ.get("DSATT_QSPLIT2", "2")),  # later pair-pairs
    "TAILSPLIT": int(_os.environ.get("DSATT_TAILSPLIT", "1")),
    "BCAST_NORM": int(_os.environ.get("DSATT_BCAST_NORM", "1")),
    # legalize multi-wait matmuls via event semaphores instead of bacc's
    # move_matmul_waits_to_ldweights pass (HW-validated variant)
    "SELF_LOAD": int(_os.environ.get("DSATT_SELF_LOAD", "1")),
}


def _compile_no_ldw_split(nc):
    """bacc.Bacc.compile() minus move_matmul_waits_to_ldweights: keeps
    matmuls self-loading; generate_event_semaphores legalizes waits."""
    from concourse import inst_simplify

    nc.insert_bir_kernel_barrier_sem_inc()
    nc.generate_event_semaphores()
    nc.remove_dead_instructions_after_branch()
    nc.validate_blocks()
    nc.dce_regs()
    nc.thread_jumps()
    nc.remove_dead_blocks()
    nc.remove_dead_allocations()
    nc.verify_switch_hints()
    nc.alloc_regs()
    inst_simplify.simplify(nc)
    nc.fuse_regops()
    nc.fuse_blocks()
    nc.replace_nops_with_events()
    for engine in nc.engines:
        nc.fuse_nops(engine)
    nc.remove_dead_nops()
    nc.remove_dangling_data()
    nc.generate_event_semaphores()
    nc.insert_library_loads()
    nc.insert_act_table_loads()
    nc.insert_hostgen_rebases()
    nc.codegen_inst_isa_subclasses()


def _emit(tc, qt, kt, v, tau4, deltat, out):
    nc = tc.nc
    Exp = mybir.ActivationFunctionType.Exp
    from contextlib import ExitStack

    PIECE = CFG["PIECE"]
    JBATCH = CFG["JBATCH"]
    QSPLIT = CFG["QSPLIT"]

    ctx = ExitStack()
    const = ctx.enter_context(tc.tile_pool(name="const", bufs=1))
    qk_pool = ctx.enter_context(tc.tile_pool(name="qk", bufs=2))
    qkb_pool = ctx.enter_context(tc.tile_pool(name="qkb", bufs=2))
    v_pool = ctx.enter_context(tc.tile_pool(name="vp", bufs=2))
    vb_pool = ctx.enter_context(tc.tile_pool(name="vbp", bufs=2))
    at_pool = ctx.enter_context(tc.tile_pool(name="atp", bufs=2))
    ob_pool = ctx.enter_context(tc.tile_pool(name="obp", bufs=3))
    ps_pool = ctx.enter_context(tc.tile_pool(name="psp", bufs=CFG["PS_BUFS"], space="PSUM"))
    po_pool = ctx.enter_context(tc.tile_pool(name="pop", bufs=2, space="PSUM"))

    # ---- one-time setup -------------------------------------------------
    # Broadcast 0.125*tau[p] to all 128 partitions via a K=1 matmul.
    c0125 = const.tile([1, 128], F32)
    nc.vector.memset(c0125[:], 0.125)
    tau_sb = const.tile([1, PPC], F32)
    nc.sync.dma_start(tau_sb[:], tau4[:])
    ptau = po_pool.tile([128, JBATCH * (D + 1)], F32, tag="po")
    nc.tensor.matmul(ptau[:, 0:PPC], lhsT=c0125[:], rhs=tau_sb[:], start=True, stop=True)
    tau_cols = const.tile([128, PPC], F32)
    nc.vector.tensor_copy(tau_cols[:], ptau[:, 0:PPC])

    # bias columns: 0.125 * delta^T  ([128, PPC*NT], column p*NT+c)
    dts = const.tile([128, PPC * NT], F32)
    nc.sync.dma_start(dts[:], deltat[:])
    bias_all = const.tile([128, PPC * NT], F32)
    nc.vector.tensor_scalar_mul(bias_all[:], dts[:], 0.125)

    for pair in range(PPC):
        pp, half = divmod(pair, 2)
        prow = slice(64 * half, 64 * half + 64)

        # ---- loads + casts (two pairs share one 128-partition tile) ----
        # Split into column pieces so the first QK matmuls start early.
        if half == 0:
            qf = qk_pool.tile([128, L], F32, tag="qf")
            kf = qk_pool.tile([128, L], F32, tag="kf")
            qb = qkb_pool.tile([128, L], BF16, tag="qb")
            kb = qkb_pool.tile([128, L], BF16, tag="kb")
            qdram = qt[2 * pp : 2 * pp + 2].rearrange("a e l -> (a e) l")
            kdram = kt[2 * pp : 2 * pp + 2].rearrange("a e l -> (a e) l")
            nsplit = QSPLIT if pp == 0 else CFG["QSPLIT2"]
            w = L // nsplit
            # pair 0 runs its chunks high-c-first (they need only the tail
            # columns of q/k), so load pieces tail-first to start PE early
            order = range(nsplit - 1, -1, -1) if pp == 0 else range(nsplit)
            for i in order:
                s = slice(w * i, w * (i + 1))
                nc.sync.dma_start(kf[:, s], kdram[:, s])
                nc.vector.tensor_copy(kb[:, s], kf[:, s])
                nc.sync.dma_start(qf[:, s], qdram[:, s])
                nc.vector.tensor_copy(qb[:, s], qf[:, s])
            _cur_qb, _cur_kb = qb, kb
        else:
            qb, kb = _cur_qb, _cur_kb  # noqa: F821

        # ---- v prep: bf16 cast + ones column -> [128, NT*(D+1)] -------
        vf = v_pool.tile([128, NT * D], F32, tag="vf")
        nc.sync.dma_start(
            vf.rearrange("q (c d) -> q c d", d=D),
            v[pair].rearrange("(c q) d -> q c d", q=128),
        )
        vb = vb_pool.tile([128, NT * (D + 1)], BF16, tag="vb")
        vb3 = vb.rearrange("q (c x) -> q c x", x=D + 1)
        nc.vector.memset(vb3[:, :, D : D + 1], 1.0)
        nc.vector.tensor_copy(vb3[:, :, 0:D], vf.rearrange("q (c d) -> q c d", d=D))

        # ---- AV batch boundaries ---------------------------------------
        batches = []
        j0_ = 0
        while j0_ < NT:
            nj_ = min(JBATCH, NT - j0_)
            # split the very last batch of the last pair for a shorter tail
            if CFG["TAILSPLIT"] and pair == PPC - 1 and j0_ + nj_ == NT and nj_ > 1:
                batches.append((j0_, nj_ - 1))
                batches.append((j0_ + nj_ - 1, 1))
            else:
                batches.append((j0_, nj_))
            j0_ += nj_

        # ---- per s-chunk: QK^T -> exp -> causal-zero; AV in j-batches --
        # pair 0 processes chunks high-c-first (needs only tail q/k pieces)
        ats = [None] * NT
        done = set()
        fired = set()
        chunk_iter = range(NT - 1, -1, -1) if pair == 0 else range(NT)
        for c in chunk_iter:
            ext = L - 128 * c
            atc = at_pool.tile([128, ext], BF16, tag=f"at{c}", name=f"at{c}_{pair}")
            ats[c] = atc
            l0 = 128 * c
            while l0 < L:
                w = min(PIECE, L - l0)
                pst = ps_pool.tile([128, PIECE], F32, tag="ps", name=f"ps_{pair}_{c}_{l0}")
                for s0 in range(0, w, MMW):
                    sw = min(MMW, w - s0)
                    nc.tensor.matmul(
                        pst[:, s0 : s0 + sw],
                        lhsT=kb[prow, 128 * c : 128 * (c + 1)],
                        rhs=qb[prow, l0 + s0 : l0 + s0 + sw],
                        start=True,
                        stop=True,
                    )
                off = l0 - 128 * c
                nc.scalar.activation(
                    atc[:, off : off + w],
                    pst[:, 0:w],
                    Exp,
                    bias=bias_all[:, NT * pair + c : NT * pair + c + 1],
                    scale=tau_cols[:, pair : pair + 1],
                )
                l0 += w
            # zero the strictly-lower triangle (s > l) of the diagonal block
            nc.gpsimd.affine_select(
                out=atc[:, 0:128],
                in_=atc[:, 0:128],
                compare_op=mybir.AluOpType.is_ge,
                fill=0.0,
                base=0,
                pattern=[[1, 128]],
                channel_multiplier=-1,
            )

            # flush AV batches as soon as all their chunks are available
            done.add(c)
            for j0, nj in batches:
                need = j0 + nj - 1
                if (j0, nj) in fired or not all(cc in done for cc in range(need + 1)):
                    continue
                fired.add((j0, nj))
                pot = po_pool.tile([128, JBATCH * (D + 1)], F32, tag="po", name=f"po_{pair}_{j0}")
                for jl in range(nj):
                    j = j0 + jl
                    for cc in range(j + 1):
                        nc.tensor.matmul(
                            pot[:, 65 * jl : 65 * jl + 65],
                            lhsT=ats[cc][:, 128 * (j - cc) : 128 * (j - cc) + 128],
                            rhs=vb3[:, cc, :],
                            start=(cc == 0),
                            stop=(cc == j),
                        )
                pot3 = pot.rearrange("q (jl x) -> q jl x", x=D + 1)
                rec = ob_pool.tile([128, JBATCH], F32, tag="rec", name=f"rec_{pair}_{j0}")
                nc.vector.reciprocal(rec[:, 0:nj], pot3[:, 0:nj, D])
                ob = ob_pool.tile([128, JBATCH * D], F32, tag="ob", name=f"ob_{pair}_{j0}")
                ob3 = ob.rearrange("q (jl d) -> q jl d", d=D)
                if CFG.get("BCAST_NORM", 1):
                    in0b, in1b = bass.broadcast_tensor_aps(
                        pot3[:, 0:nj, 0:D], rec[:, 0:nj].unsqueeze(2)
                    )
                    nc.vector.tensor_tensor(
                        out=ob3[:, 0:nj, :], in0=in0b, in1=in1b,
                        op=mybir.AluOpType.mult,
                    )
                else:
                    for jl in range(nj):
                        nc.vector.tensor_scalar_mul(
                            ob[:, D * jl : D * jl + D],
                            pot3[:, jl, 0:D],
                            rec[:, jl : jl + 1],
                        )
                nc.sync.dma_start(
                    out[pair, 128 * j0 : 128 * (j0 + nj), :].rearrange(
                        "(jl q) d -> q jl d", q=128
                    ),
                    ob.rearrange("q (jl d) -> q jl d", d=D)[:, 0:nj, :],
                )
    ctx.close()


_NC_CACHE = {}


def _get_nc():
    if "nc" not in _NC_CACHE:
        nc = bacc.Bacc("TRN2", target_bir_lowering=False, debug=False)
        qt = nc.dram_tensor("qt", [PPC, E, L], F32, kind="ExternalInput")
        kt = nc.dram_tensor("kt", [PPC, E, L], F32, kind="ExternalInput")
        v = nc.dram_tensor("v", [PPC, L, D], F32, kind="ExternalInput")
        tau4 = nc.dram_tensor("tau4", [1, PPC], F32, kind="ExternalInput")
        deltat = nc.dram_tensor("deltat", [128, PPC * NT], F32, kind="ExternalInput")
        out = nc.dram_tensor("out", [PPC, L, D], F32, kind="ExternalOutput")
        with tile.TileContext(nc) as tc:
            _emit(tc, qt.ap(), kt.ap(), v.ap(), tau4.ap(), deltat.ap(), out.ap())
        if CFG["SELF_LOAD"]:
            _compile_no_ldw_split(nc)
        else:
            nc.compile()
        _NC_CACHE["nc"] = nc
    return _NC_CACHE["nc"]


def _host_prep(queries, keys, values, tau, delta):
    """Shard + lay out full inputs into 8 per-core input maps."""
    queries = np.asarray(queries, np.float32)
    keys = np.asarray(keys, np.float32)
    values = np.asarray(values, np.float32)
    qT = np.ascontiguousarray(queries.transpose(0, 2, 3, 1)).reshape(PAIRS, E, L)
    kT = np.ascontiguousarray(keys.transpose(0, 2, 3, 1)).reshape(PAIRS, E, L)
    vv = np.ascontiguousarray(values.transpose(0, 2, 1, 3)).reshape(PAIRS, L, D)
    tau_flat = np.asarray(tau, np.float32).reshape(B)
    # delta^T per batch: [128, NT] where column c = delta[b, 128c:128c+128]
    dT = np.ascontiguousarray(
        np.asarray(delta, np.float32).reshape(B, NT, 128).transpose(0, 2, 1)
    )
    in_maps = []
    for m in range(NCORES):
        gs = range(PPC * m, PPC * (m + 1))
        bidx = [g // H for g in gs]
        in_maps.append(
            {
                "qt": np.ascontiguousarray(qT[PPC * m : PPC * (m + 1)]),
                "kt": np.ascontiguousarray(kT[PPC * m : PPC * (m + 1)]),
                "v": np.ascontiguousarray(vv[PPC * m : PPC * (m + 1)]),
                "tau4": tau_flat[bidx].reshape(1, PPC).copy(),
                "deltat": np.concatenate([dT[b] for b in bidx], axis=1),
            }
        )
    return in_maps


def _host_gather(per_core_outs):
    full = np.stack(per_core_outs).reshape(B, H, L, D)
    return np.ascontiguousarray(full.transpose(0, 2, 1, 3))


def kernel(queries, keys, values, tau, delta, **_):
    nc = _get_nc()
    in_maps = _host_prep(queries, keys, values, tau, delta)
    res = run_bass_kernel_spmd(nc, in_maps, list(range(NCORES)))
    return _host_gather([res.results[m]["out"] for m in range(NCORES)])
